# revision 14
# baseline (speedup 1.0000x reference)
"""Swin-style windowed multi-head attention on 8 Trainium2 NeuronCores.

Problem: nn_Attention_86792699118108
  x [16, 3136, 768], 56x56 spatial, window 14x14 (no padding needed),
  12 heads, head_dim 64. 256 independent windows -> 32 windows per core.

Strategy (data-parallel over windows):
  host: window-partition x, cast to bf16, pre-transpose/scale weights,
        pre-gather the relative-position bias table (static indices) and
        exponentiate it (exp(s + b) == exp(s) * exp(b)).
  device (per core, SPMD):
    xT (chan-major) via DMA-transpose ->
    qkv GEMM (q,k head-dim-major; v token-major with interleaved ones col) ->
    scores_T = k.T q per head (PE) -> exp (ACT, no max-subtraction: scores
    are provably small) -> * exp(rpb) (DVE) ->
    AV: out_T[d, n] with ones row giving softmax sums for free ->
    normalize on eviction (gpsimd partition-broadcast of 1/sums) ->
    proj GEMM -> token-major f32 out -> DMA.
  host: window-reverse.
"""

import numpy as np
import ml_dtypes

WS = 14
NH = 12
HD = 64
C = 768
N = WS * WS  # 196 tokens per window
NCORES = 8

_BF16 = ml_dtypes.bfloat16

_prog_cache = {}


def _rel_index(ws):
    coords = np.stack(np.meshgrid(np.arange(ws), np.arange(ws), indexing="ij"))
    cf = coords.reshape(2, -1)
    rel = (cf[:, :, None] - cf[:, None, :]).transpose(1, 2, 0).astype(np.int64)
    rel[..., 0] += ws - 1
    rel[..., 1] += ws - 1
    rel[..., 0] *= 2 * ws - 1
    return rel.sum(-1)


def _build_program(n_win):
    import concourse.bass as bass
    import concourse.mybir as mybir
    import concourse.tile as tile
    from concourse import bacc
    from contextlib import ExitStack

    assert n_win % 4 == 0
    n_grp = n_win // 4
    n_tok = n_win * N

    BF = mybir.dt.bfloat16
    F32 = mybir.dt.float32
    AF = mybir.ActivationFunctionType

    MC = [(0, 128), (128, 68)]  # token/key chunks within a 196-token window

    nc = bacc.Bacc("TRN2", target_bir_lowering=False, debug=False,
                   num_devices=NCORES)

    x = nc.dram_tensor("x", [n_tok, C], BF, kind="ExternalInput")
    wqkvT = nc.dram_tensor("wqkvT", [C, 3 * C], BF, kind="ExternalInput")
    wpT = nc.dram_tensor("wpT", [C, C], BF, kind="ExternalInput")
    er = nc.dram_tensor("er", [N, NH * N], BF, kind="ExternalInput")
    qkb = nc.dram_tensor("qkb", [128, 12], F32, kind="ExternalInput")
    vb = nc.dram_tensor("vb", [1, C], F32, kind="ExternalInput")
    pb = nc.dram_tensor("pb", [1, C], F32, kind="ExternalInput")
    y = nc.dram_tensor("y", [n_tok, C], F32, kind="ExternalOutput")

    def bcast_ap(handle, p):
        a = handle[:, :]
        return bass.AP(tensor=a.tensor, offset=a.offset, ap=[[0, p], [1, C]])

    with ExitStack() as ctx:
        tc = ctx.enter_context(tile.TileContext(nc))
        consts = ctx.enter_context(tc.tile_pool(name="consts", bufs=1))
        grp = ctx.enter_context(tc.tile_pool(name="grp", bufs=2))
        win = ctx.enter_context(tc.tile_pool(name="win", bufs=2))
        # One shared psum pool: every tile <= 2 banks ([*, <=1024] f32),
        # 4 slots = all 8 banks. NOTE: hardware allows only ONE matmul
        # accumulation group per psum BANK at a time, hence the 512-column
        # stride for multi-head tiles.
        psp = ctx.enter_context(tc.tile_pool(name="psp", bufs=2, space="PSUM"))

        # ---- constants -------------------------------------------------
        wq = []
        for ic in range(6):
            t = consts.tile([128, 3 * C], BF, tag=f"wq{ic}", name=f"wq{ic}")
            nc.sync.dma_start(out=t, in_=wqkvT[ic * 128:(ic + 1) * 128, :])
            wq.append(t)
        wp = []
        for ic in range(6):
            t = consts.tile([128, C], BF, tag=f"wp{ic}", name=f"wp{ic}")
            nc.sync.dma_start(out=t, in_=wpT[ic * 128:(ic + 1) * 128, :])
            wp.append(t)
        er_t = []
        for mci, (mo, msz) in enumerate(MC):
            t = consts.tile([msz, NH * N], BF, tag=f"er{mci}", name=f"er{mci}")
            nc.sync.dma_start(out=t, in_=er[mo:mo + msz, :])
            er_t.append(t)
        qkb_t = consts.tile([128, 12], F32, tag="qkb", name="qkb_t")
        nc.sync.dma_start(out=qkb_t, in_=qkb[:, :])
        vb_t = consts.tile([128, C], F32, tag="vb", name="vb_t")
        nc.gpsimd.dma_start(out=vb_t, in_=bcast_ap(vb, 128))
        pb_t = consts.tile([128, C], F32, tag="pb", name="pb_t")
        nc.gpsimd.dma_start(out=pb_t, in_=bcast_ap(pb, 128))

        for g in range(n_grp):
            t0 = g * 4 * N  # 784 tokens per group

            # ---- x transpose-load: xT[ic] = [cin 128, tok 784] ---------
            xT = []
            for ic in range(6):
                t = grp.tile([128, 4 * N], BF, tag=f"xT{ic}", name=f"xT{ic}")
                nc.sync.dma_start(
                    out=t,
                    in_=x[t0:t0 + 4 * N, ic * 128:(ic + 1) * 128],
                    transpose=True)
                xT.append(t)

            # ---- q, k: head-dim-major [cout 128, tok 784] --------------
            qk = [grp.tile([128, 4 * N], BF, tag=f"qk{oc}", name=f"qk{oc}") for oc in range(12)]
            for oc in range(12):
                pss = [psp.tile([128, 392], F32, tag="ps", name="psmm", bufs=4)
                       for _ in range(2)]
                for ic in range(6):
                    for s in range(2):  # same lhsT back-to-back for both spans
                        nc.tensor.matmul(
                            pss[s],
                            wq[ic][:, oc * 128:(oc + 1) * 128],
                            xT[ic][:, s * 392:(s + 1) * 392],
                            start=(ic == 0), stop=(ic == 5))
                for s in range(2):
                    nc.vector.tensor_scalar_add(
                        qk[oc][:, s * 392:(s + 1) * 392], pss[s],
                        qkb_t[:, oc:oc + 1])

            # ---- v: token-major, interleaved [64 v-cols + ones] per head
            v_t = {}
            for w4 in range(4):
                for mci, (mo, msz) in enumerate(MC):
                    vt = grp.tile([128, NH * 65], BF, tag=f"v{w4}_{mci}", name=f"v{w4}_{mci}")
                    vr = vt.rearrange("p (h e) -> p h e", e=65)
                    pss = [psp.tile([128, 384], F32, tag="ps", name="psmm", bufs=4)
                           for _ in range(2)]
                    for ic in range(6):
                        for half in range(2):  # same lhsT for both halves
                            nc.tensor.matmul(
                                pss[half][:msz],
                                xT[ic][:, w4 * N + mo: w4 * N + mo + msz],
                                wq[ic][:, 1536 + half * 384: 1536 + (half + 1) * 384],
                                start=(ic == 0), stop=(ic == 5))
                    for half in range(2):
                        nc.vector.tensor_add(
                            vr[:msz, half * 6:(half + 1) * 6, 0:64],
                            pss[half][:msz].rearrange("p (h e) -> p h e", e=64),
                            vb_t[:msz, half * 384:(half + 1) * 384]
                                .rearrange("p (h e) -> p h e", e=64))
                    nc.vector.memset(vr[:msz, :, 64:65], 1.0)
                    v_t[(w4, mci)] = vt

            # ---- attention + proj per window ---------------------------
            for w4 in range(4):
                w0 = w4 * N

                # exp(scores_T) per m-chunk, all heads wide: [msz, 12*196]
                ex = []
                for mci, (mo, msz) in enumerate(MC):
                    ex.append(win.tile([msz, NH * N], BF, tag=f"ex{mci}", name=f"ex{mci}"))
                for hg in range(6):  # head pairs; one accum group per bank
                    for mci, (mo, msz) in enumerate(MC):
                        ps = psp.tile([128, 1024], F32, tag="ps", name="psqk", bufs=4)
                        for j in range(2):
                            h = hg * 2 + j
                            ro = (h % 2) * 64
                            nc.tensor.matmul(
                                ps[:msz, j * 512:j * 512 + N],
                                qk[6 + h // 2][ro:ro + 64, w0 + mo: w0 + mo + msz],
                                qk[h // 2][ro:ro + 64, w0:w0 + N],
                                start=True, stop=True)
                        nc.scalar.activation(
                            ex[mci].rearrange("p (h n) -> p h n", n=N)
                                [:, hg * 2:(hg + 1) * 2, :],
                            ps[:msz].rearrange("p (h n) -> p h n", n=512)
                                [:, :, 0:N],
                            AF.Exp)

                # unnormalized attn_T = exp(scores_T) * exp(rpb_T)
                attn = []
                for mci, (mo, msz) in enumerate(MC):
                    at = win.tile([msz, NH * N], BF, tag=f"attn{mci}", name=f"attn{mci}")
                    nc.vector.tensor_mul(at, ex[mci], er_t[mci])
                    attn.append(at)

                # AV: out_T[65, n] per head; row 64 = softmax sums
                aoT = [win.tile([128, N], BF, tag=f"aoT{i}", name=f"aoT{i}") for i in range(6)]
                for p6 in range(6):
                    ps = psp.tile([65, 1024], F32, tag="ps", name="psav", bufs=4)
                    for j in range(2):
                        h = p6 * 2 + j
                        for mci, (mo, msz) in enumerate(MC):
                            nc.tensor.matmul(
                                ps[:, j * 512:j * 512 + N],
                                v_t[(w4, mci)][:msz, h * 65:(h + 1) * 65],
                                attn[mci][:, h * N:(h + 1) * N],
                                start=(mci == 0), stop=(mci == 1))
                    sm = win.tile([1, 2 * N], F32, tag="sums", name="sm")
                    nc.scalar.activation(
                        sm.rearrange("p (j n) -> p j n", n=N),
                        ps[64:65].rearrange("p (j n) -> p j n", n=512)[:, :, 0:N],
                        AF.Copy)
                    rr = win.tile([1, 2 * N], F32, tag="recr", name="recr")
                    nc.vector.reciprocal_approx_fast(rr, sm)
                    rrep = win.tile([128, 2 * N], F32, tag="rrep", name="rrep")
                    nc.gpsimd.partition_broadcast(rrep, rr)
                    for j in range(2):
                        h = p6 * 2 + j
                        nc.vector.tensor_mul(
                            aoT[p6][(h % 2) * 64:(h % 2) * 64 + 64, :],
                            ps[0:64, j * 512:j * 512 + N],
                            rrep[0:64, j * N:(j + 1) * N])

                # proj: y[tok, c] = attn_out @ proj_w.T + proj_b
                ysb = [win.tile([128, C], F32, tag=f"ysb{i}", name=f"ysb{i}") for i in range(2)]
                for mci, (mo, msz) in enumerate(MC):
                    pss = [psp.tile([128, 384], F32, tag="ps", name="psmm", bufs=4)
                           for _ in range(2)]
                    for ic in range(6):
                        for half in range(2):  # same lhsT for both halves
                            nc.tensor.matmul(
                                pss[half][:msz],
                                aoT[ic][:, mo:mo + msz],
                                wp[ic][:, half * 384:(half + 1) * 384],
                                start=(ic == 0), stop=(ic == 5))
                    for half in range(2):
                        nc.vector.tensor_add(
                            ysb[mci][:msz, half * 384:(half + 1) * 384],
                            pss[half][:msz],
                            pb_t[:msz, half * 384:(half + 1) * 384])
                    nc.sync.dma_start(
                        out=y[t0 + w0 + mo: t0 + w0 + mo + msz, :],
                        in_=ysb[mci][:msz, :])

    nc.compile()
    return nc


def _get_program(n_win):
    if n_win not in _prog_cache:
        _prog_cache[n_win] = _build_program(n_win)
    return _prog_cache[n_win]


def _host_prep(x, qkv_w, q_bias, v_bias, rel_bias_table, proj_w, proj_b, H, W):
    B = x.shape[0]
    nws = H // WS  # windows per side
    xw = (np.asarray(x, np.float32)
          .reshape(B, nws, WS, nws, WS, C)
          .transpose(0, 1, 3, 2, 4, 5)
          .reshape(-1, N, C))  # [Bw, 196, C]

    scale = HD ** -0.5
    wq_s = np.array(qkv_w, np.float32, copy=True)
    wq_s[0:C] *= scale
    wqkvT = np.ascontiguousarray(wq_s.T).astype(_BF16)
    wpT = np.ascontiguousarray(np.asarray(proj_w, np.float32).T).astype(_BF16)

    idx = _rel_index(WS).reshape(-1)
    rpb = np.asarray(rel_bias_table, np.float32)[idx].reshape(N, N, NH)  # [n,m,h]
    er = np.ascontiguousarray(
        np.exp(rpb).transpose(1, 2, 0).reshape(N, NH * N)).astype(_BF16)

    qkv_b = np.concatenate([
        np.asarray(q_bias, np.float32) * scale,
        np.zeros(C, np.float32),
        np.asarray(v_bias, np.float32)])
    qkb = np.ascontiguousarray(qkv_b[0:2 * C].reshape(12, 128).T)
    vbias = np.ascontiguousarray(np.asarray(v_bias, np.float32).reshape(1, C))
    pbias = np.ascontiguousarray(np.asarray(proj_b, np.float32).reshape(1, C))

    xbf = np.ascontiguousarray(xw.reshape(-1, C)).astype(_BF16)
    return xbf, wqkvT, wpT, er, qkb, vbias, pbias


def kernel(x, qkv_w, q_bias, v_bias, rel_bias_table, proj_w, proj_b, H, W,
           _return_results=False):
    from concourse.bass_utils import run_bass_kernel_spmd

    x = np.asarray(x)
    B = x.shape[0]
    H = int(H)
    W = int(W)
    nws = H // WS

    xbf, wqkvT, wpT, er, qkb, vbias, pbias = _host_prep(
        x, qkv_w, q_bias, v_bias, rel_bias_table, proj_w, proj_b, H, W)

    Bw = B * nws * nws
    n_win_core = Bw // NCORES
    nc = _get_program(n_win_core)

    tok_core = n_win_core * N
    in_maps = []
    for c in range(NCORES):
        in_maps.append({
            "x": xbf[c * tok_core:(c + 1) * tok_core],
            "wqkvT": wqkvT, "wpT": wpT, "er": er,
            "qkb": qkb, "vb": vbias, "pb": pbias,
        })

    res = run_bass_kernel_spmd(nc, in_maps, list(range(NCORES)))
    yw = np.concatenate([res.results[c]["y"] for c in range(NCORES)], axis=0)
    out = (yw.reshape(B, nws, nws, WS, WS, C)
           .transpose(0, 1, 3, 2, 4, 5)
           .reshape(B, H * W, C).astype(np.float32))
    if _return_results:
        return out, res
    return out
